# revision 4
# baseline (speedup 1.0000x reference)
# GQA attention layer (B=1, S=2048, HID=2560, H=32, HKV=8, D=128) on 8 TRN2
# NeuronCores. Tensor-parallel over kv-head groups: core c owns kv head c and
# its 4 query heads (Wq/Wk/Wv row shards, Wo column shard). The o_proj
# partials are combined with an on-device ReduceScatter over the sequence
# axis; the host reassembles the sequence-sharded outputs.
#
# Per-core dataflow (all matmuls bf16 -> fp32 PSUM):
#   1. QKV projection from X^T tiles (s-major output layout), per-head
#      RMSNorm + RoPE on DVE, PE-transpose of Q/K into [d, s] layout.
#   2. Scores are computed transposed (S^T[k, q] = K Q^T) so that the
#      P^T @ V matmul needs no transpose of the 16.8M-element prob matrix.
#      exp() on the scalar engine in 2-PSUM-bank batches (no max
#      subtraction: |scores| is bounded). Scores and PV matmuls are
#      software-pipelined so the PE never waits on the exp stream.
#      Softmax denominators: pairwise kt-tree accumulation of the exp
#      tiles on DVE, then a single all-ones [128,128] stationary matmul
#      per unit broadcasts the partition-sums to every output row; one
#      DVE reciprocal + multiply normalizes the PV output.
#   3. o_proj per 512-row chunk (DVE evictions only - the scalar engine
#      stays exp-only in phase 2, avoiding ACT table reloads). The first
#      3 chunks ReduceScatter at 512 rows; the last chunk scatters per
#      128-row subtile so the final RS tail is ~4x shorter.
import sys

if "/opt/trn_rl_repo" not in sys.path:
    sys.path.insert(0, "/opt/trn_rl_repo")

import numpy as np
import ml_dtypes

import concourse.bacc as bacc
import concourse.mybir as mybir
import concourse.tile as tile
from concourse import bass_utils, masks

BF16 = mybir.dt.bfloat16
F32 = mybir.dt.float32

B, S, HID = 1, 2048, 2560
H, HKV, D = 32, 8, 128
G = H // HKV  # q heads per kv head (= per core)
NC = 8  # cores
DQ = G * D  # per-core q width (512)
EPS = 1e-6
SCALE = 1.0 / float(np.sqrt(D))

ST = 128          # s positions per compute tile
N_ST = S // ST    # 16
HC = HID // 128   # 20 contraction chunks
XL = 256          # s positions per X^T DMA load tile
N_XL = S // XL    # 8
QC = 512          # q positions per attention unit
N_QC = S // QC    # 4
N_KT = S // 128   # 16 k tiles per attention unit
NP = N_KT // 2    # 8 score/exp pairs per unit
NO = HID // 512   # 5 o_proj free-dim chunks
MR = 128          # rows per mini-ReduceScatter (last chunk)

_NC_CACHE = None


def _build(reps: int = 1, single: bool = False):
    nc = bacc.Bacc(
        "TRN2", target_bir_lowering=False, debug=False,
        num_devices=(1 if single else NC),
    )

    xt_d = nc.dram_tensor("xt", [N_XL, HC, 128, XL], BF16, kind="ExternalInput").ap()
    wq_d = nc.dram_tensor("wq", [HC, 128, DQ], BF16, kind="ExternalInput").ap()
    wkv_d = nc.dram_tensor("wkv", [HC, 128, 2 * D], BF16, kind="ExternalInput").ap()
    wo_d = nc.dram_tensor("wo", [G, 128, HID], BF16, kind="ExternalInput").ap()
    cwq_d = nc.dram_tensor("cwq", [N_ST, 128, D], F32, kind="ExternalInput").ap()
    swq_d = nc.dram_tensor("swq", [N_ST, 128, D], F32, kind="ExternalInput").ap()
    cwk_d = nc.dram_tensor("cwk", [N_ST, 128, D], F32, kind="ExternalInput").ap()
    swk_d = nc.dram_tensor("swk", [N_ST, 128, D], F32, kind="ExternalInput").ap()
    out_d = nc.dram_tensor("out", [S // NC, HID], BF16, kind="ExternalOutput").ap()

    with tile.TileContext(nc) as tc:
        with (
            tc.tile_pool(name="const", bufs=1) as cpool,
            tc.tile_pool(name="xt", bufs=2) as xt_pool,
            tc.tile_pool(name="cs", bufs=8) as cs_pool,
            tc.tile_pool(name="qw", bufs=5) as qw_pool,
            tc.tile_pool(name="kw", bufs=6) as kw_pool,
            tc.tile_pool(name="ro", bufs=2) as ro_pool,
            tc.tile_pool(name="sm", bufs=4) as sm_pool,
            tc.tile_pool(name="ep", bufs=3) as ep_pool,
            tc.tile_pool(name="tr", bufs=2) as tr_pool,
            tc.tile_pool(name="ot", bufs=8) as ot_pool,
            tc.tile_pool(name="ob", bufs=2) as ob_pool,
            tc.tile_pool(name="psA", bufs=2, space="PSUM") as psA,
            tc.tile_pool(name="psB", bufs=2, space="PSUM") as psB,
            tc.tile_pool(name="psC", bufs=2, space="PSUM") as psC,
            tc.tile_pool(name="dram", bufs=1, space="DRAM") as dram,
        ):
            for _rep in range(reps):
                # ---- resident constants / weights ----
                ident = cpool.tile([128, 128], BF16, tag="ident")
                masks.make_identity(nc, ident[:])
                # all-ones stationary: one sums matmul per unit yields the
                # softmax denominator replicated across all 128 partitions
                ones_k = cpool.tile([128, 128], BF16, tag="ones_k")
                nc.vector.memset(ones_k[:], 1.0)

                # interleave per-chunk weight + xt[0] chunk loads so the
                # first st=0 matmuls start as soon as chunk 0 lands
                xt_t = xt_pool.tile([128, HC, XL], BF16, tag="xt")
                wq_t = []
                wkv_t = []
                for ch in range(HC):
                    w1 = cpool.tile([128, DQ], BF16, tag=f"wq{ch}")
                    nc.sync.dma_start(w1[:], wq_d[ch])
                    wq_t.append(w1)
                    w2 = cpool.tile([128, 2 * D], BF16, tag=f"wkv{ch}")
                    nc.sync.dma_start(w2[:], wkv_d[ch])
                    wkv_t.append(w2)
                    nc.sync.dma_start(xt_t[:, ch, :], xt_d[0, ch])
                xt_next = xt_pool.tile([128, HC, XL], BF16, tag="xt")
                nc.sync.dma_start(xt_next[:], xt_d[1].rearrange("c p s -> p c s"))

                qt_sb = cpool.tile([128, G, S], BF16, tag="qt")   # Q^T  [d, h, s]
                kt_sb = cpool.tile([128, S], BF16, tag="kt")      # K^T  [d, s]
                v_sb = cpool.tile([128, N_KT, D], BF16, tag="v")  # V    [s%128, kt, d]

                # ================= phase 1: QKV + norm + rope + transpose ======
                for st in range(N_ST):
                    if st % (XL // ST) == 0 and st > 0:
                        if st // (XL // ST) == 1:
                            xt_t = xt_next
                        else:
                            xt_t = xt_pool.tile([128, HC, XL], BF16, tag="xt")
                            nc.sync.dma_start(
                                xt_t[:],
                                xt_d[st // (XL // ST)].rearrange("c p s -> p c s"),
                            )
                    soff = (st % (XL // ST)) * ST

                    cwq_t = cs_pool.tile([128, D], F32, tag="cs")
                    nc.sync.dma_start(cwq_t[:], cwq_d[st])
                    swq_t = cs_pool.tile([128, D], F32, tag="cs")
                    nc.sync.dma_start(swq_t[:], swq_d[st])
                    cwk_t = cs_pool.tile([128, D], F32, tag="cs")
                    nc.sync.dma_start(cwk_t[:], cwk_d[st])
                    swk_t = cs_pool.tile([128, D], F32, tag="cs")
                    nc.sync.dma_start(swk_t[:], swk_d[st])

                    # q in bank 0, kv in the low half of bank 1
                    qkv_ps = psA.tile([128, 2, DQ], F32, tag="a")
                    for ch in range(HC):
                        lhs = xt_t[:, ch, soff : soff + ST]
                        nc.tensor.matmul(
                            qkv_ps[:, 0, :], lhs, wq_t[ch][:],
                            start=(ch == 0), stop=(ch == HC - 1),
                        )
                        nc.tensor.matmul(
                            qkv_ps[:, 1, 0 : 2 * D], lhs, wkv_t[ch][:],
                            start=(ch == 0), stop=(ch == HC - 1),
                        )

                    # evictions (scalar engine; phase 1 is Copy-table only)
                    q_sb = qw_pool.tile([128, DQ], F32, tag="qw")
                    nc.scalar.copy(q_sb[:], qkv_ps[:, 0, :])
                    k_sb = kw_pool.tile([128, D], F32, tag="kw")
                    nc.scalar.copy(k_sb[:], qkv_ps[:, 1, 0:D])
                    nc.scalar.copy(v_sb[:, st, :], qkv_ps[:, 1, D : 2 * D])

                    # ---- RMSNorm (per head) ----
                    sq = qw_pool.tile([128, DQ], F32, tag="qw")
                    nc.vector.tensor_mul(sq[:], q_sb[:], q_sb[:])
                    ssq = sm_pool.tile([128, G + 1], F32, tag="sm")
                    nc.vector.tensor_reduce(
                        ssq[:, 0:G], sq[:].rearrange("p (h d) -> p h d", d=D),
                        axis=mybir.AxisListType.X, op=mybir.AluOpType.add,
                    )
                    ksq = kw_pool.tile([128, D], F32, tag="kw")
                    nc.vector.tensor_mul(ksq[:], k_sb[:], k_sb[:])
                    nc.vector.tensor_reduce(
                        ssq[:, G : G + 1], ksq[:].unsqueeze(1),
                        axis=mybir.AxisListType.X, op=mybir.AluOpType.add,
                    )
                    var = sm_pool.tile([128, G + 1], F32, tag="sm")
                    nc.vector.tensor_scalar(
                        var[:], ssq[:], 1.0 / D, EPS,
                        op0=mybir.AluOpType.mult, op1=mybir.AluOpType.add,
                    )
                    rt = sm_pool.tile([128, G + 1], F32, tag="sm")
                    nc.scalar.sqrt(rt[:], var[:])
                    rq = sm_pool.tile([128, G + 1], F32, tag="sm")
                    nc.vector.reciprocal(rq[:], rt[:])
                    rk = rq

                    # ---- normalize + rope (DVE) ----
                    qn = qw_pool.tile([128, DQ], F32, tag="qw")
                    qn3 = qn[:].rearrange("p (h d) -> p h d", d=D)
                    nc.vector.tensor_tensor(
                        qn3, q_sb[:].rearrange("p (h d) -> p h d", d=D),
                        rq[:, 0:G].unsqueeze(2).to_broadcast([128, G, D]),
                        op=mybir.AluOpType.mult,
                    )
                    t1 = qw_pool.tile([128, DQ], F32, tag="qw")
                    t13 = t1[:].rearrange("p (h d) -> p h d", d=D)
                    cwq3 = cwq_t[:].unsqueeze(1).to_broadcast([128, G, D])
                    swq3 = swq_t[:].unsqueeze(1).to_broadcast([128, G, D])
                    nc.vector.tensor_tensor(t13, qn3, cwq3, op=mybir.AluOpType.mult)
                    u = qw_pool.tile([128, DQ], F32, tag="qw")
                    u3 = u[:].rearrange("p (h d) -> p h d", d=D)
                    hd = D // 2
                    nc.vector.tensor_tensor(
                        u3[:, :, 0:hd], qn3[:, :, hd:D], swq3[:, :, 0:hd],
                        op=mybir.AluOpType.mult,
                    )
                    nc.vector.tensor_tensor(
                        u3[:, :, hd:D], qn3[:, :, 0:hd], swq3[:, :, hd:D],
                        op=mybir.AluOpType.mult,
                    )
                    qro = ro_pool.tile([128, DQ], BF16, tag="qro")
                    qro3 = qro[:].rearrange("p (h d) -> p h d", d=D)
                    nc.vector.tensor_sub(qro3[:, :, 0:hd], t13[:, :, 0:hd], u3[:, :, 0:hd])
                    nc.vector.tensor_add(qro3[:, :, hd:D], t13[:, :, hd:D], u3[:, :, hd:D])

                    kn = kw_pool.tile([128, D], F32, tag="kw")
                    nc.vector.tensor_tensor(
                        kn[:], k_sb[:],
                        rk[:, G : G + 1].to_broadcast([128, D]),
                        op=mybir.AluOpType.mult,
                    )
                    kt1 = kw_pool.tile([128, D], F32, tag="kw")
                    nc.vector.tensor_tensor(kt1[:], kn[:], cwk_t[:], op=mybir.AluOpType.mult)
                    ku = kw_pool.tile([128, D], F32, tag="kw")
                    nc.vector.tensor_tensor(
                        ku[:, 0:hd], kn[:, hd:D], swk_t[:, 0:hd], op=mybir.AluOpType.mult
                    )
                    nc.vector.tensor_tensor(
                        ku[:, hd:D], kn[:, 0:hd], swk_t[:, hd:D], op=mybir.AluOpType.mult
                    )
                    kro = ro_pool.tile([128, D], BF16, tag="kro")
                    nc.vector.tensor_sub(kro[:, 0:hd], kt1[:, 0:hd], ku[:, 0:hd])
                    nc.vector.tensor_add(kro[:, hd:D], kt1[:, hd:D], ku[:, hd:D])

                    # ---- transpose Q heads + K into [d, s] ----
                    for h in range(G):
                        tp = psC.tile([128, 128], BF16, tag="c")
                        nc.tensor.transpose(tp[:], qro[:, h * D : (h + 1) * D], ident[:])
                        nc.scalar.copy(qt_sb[:, h, st * ST : (st + 1) * ST], tp[:])
                    tp = psC.tile([128, 128], BF16, tag="c")
                    nc.tensor.transpose(tp[:], kro[:], ident[:])
                    nc.scalar.copy(kt_sb[:, st * ST : (st + 1) * ST], tp[:])

                # ================= phase 2: attention + o_proj + RS ============
                # wo is first needed ~20us into phase 2; load it behind the
                # phase-1 traffic instead of ahead of it
                wo_sb = cpool.tile([128, G, HID], BF16, tag="wo")
                nc.sync.dma_start(wo_sb[:], wo_d.rearrange("c p n -> p c n"))
                for qc in range(N_QC):
                    ot_tiles = []
                    for h in range(G):
                        ep = ep_pool.tile([128, N_KT, QC], BF16, tag="ep")
                        qsl = qt_sb[:, h, qc * QC : (qc + 1) * QC]
                        tr1 = []

                        def emit_scores(p, ep=ep, qsl=qsl, tr1=tr1):
                            s2 = psA.tile([128, 2, QC], F32, tag="a")
                            nc.tensor.matmul(
                                s2[:, 0, :],
                                kt_sb[:, (2 * p) * 128 : (2 * p + 1) * 128],
                                qsl, start=True, stop=True,
                            )
                            nc.tensor.matmul(
                                s2[:, 1, :],
                                kt_sb[:, (2 * p + 1) * 128 : (2 * p + 2) * 128],
                                qsl, start=True, stop=True,
                            )
                            # one exp over both PSUM banks
                            nc.scalar.activation(
                                ep[:, 2 * p : 2 * p + 2, :], s2[:],
                                mybir.ActivationFunctionType.Exp, scale=SCALE,
                            )
                            # kt-tree level 1 on DVE
                            t = tr_pool.tile([128, QC], BF16, tag="t1", bufs=9)
                            nc.vector.tensor_add(t[:], ep[:, 2 * p, :], ep[:, 2 * p + 1, :])
                            tr1.append(t)

                        # software pipeline: 2 pairs of score-lead, then
                        # alternate pv(pair) / scores(pair+2) so the PE and
                        # the exp stream advance in lockstep
                        emit_scores(0)
                        emit_scores(1)
                        pv_ps = psB.tile([128, QC], F32, tag="b")
                        for p in range(NP):
                            nc.tensor.matmul(
                                pv_ps[:], v_sb[:, 2 * p, :], ep[:, 2 * p, :],
                                start=(p == 0), stop=False,
                            )
                            nc.tensor.matmul(
                                pv_ps[:], v_sb[:, 2 * p + 1, :], ep[:, 2 * p + 1, :],
                                start=False, stop=(p == NP - 1),
                            )
                            if p + 2 < NP:
                                emit_scores(p + 2)

                        # kt-tree levels 2..4 (DVE), then broadcast-sum on PE
                        tr2 = []
                        for j in range(4):
                            t = tr_pool.tile([128, QC], BF16, tag="t2", bufs=5)
                            nc.vector.tensor_add(t[:], tr1[2 * j][:], tr1[2 * j + 1][:])
                            tr2.append(t)
                        tr3 = []
                        for j in range(2):
                            t = tr_pool.tile([128, QC], BF16, tag="t3", bufs=4)
                            nc.vector.tensor_add(t[:], tr2[2 * j][:], tr2[2 * j + 1][:])
                            tr3.append(t)
                        colsum = tr_pool.tile([128, QC], BF16, tag="t4", bufs=4)
                        nc.vector.tensor_add(colsum[:], tr3[0][:], tr3[1][:])
                        sums_ps = psC.tile([128, QC], F32, tag="c")
                        nc.tensor.matmul(
                            sums_ps[:], ones_k[:], colsum[:], start=True, stop=True
                        )
                        rb = sm_pool.tile([128, QC], F32, tag="rb", bufs=2)
                        nc.vector.reciprocal(rb[:], sums_ps[:])
                        ot = ot_pool.tile([128, QC], BF16, tag="ot")
                        nc.vector.tensor_tensor(
                            ot[:], pv_ps[:], rb[:], op=mybir.AluOpType.mult
                        )
                        ot_tiles.append(ot)

                    # o_proj for this 512-row chunk. DVE evictions only.
                    RROWS = QC // NC  # 64 output rows per core per full RS
                    if qc < N_QC - 1:
                        rs_in = dram.tile([QC, HID], BF16, tag=f"rsin{qc}")
                        rs_out = dram.tile([RROWS, HID], BF16, tag=f"rsout{qc}")
                    for si in range(QC // ST):
                        if qc == N_QC - 1:
                            rs_in = dram.tile([MR, HID], BF16, tag=f"rsin3_{si}")
                            rs_out = dram.tile([MR // NC, HID], BF16, tag=f"rsout3_{si}")
                        ob = ob_pool.tile([128, HID], BF16, tag="ob")
                        for no in range(NO):
                            y_ps = psB.tile([128, 512], F32, tag="b")
                            for h in range(G):
                                nc.tensor.matmul(
                                    y_ps[:],
                                    ot_tiles[h][:, si * ST : (si + 1) * ST],
                                    wo_sb[:, h, no * 512 : (no + 1) * 512],
                                    start=(h == 0), stop=(h == G - 1),
                                )
                            nc.vector.tensor_copy(
                                ob[:, no * 512 : (no + 1) * 512], y_ps[:]
                            )
                            ro0 = 0 if qc == N_QC - 1 else si * ST
                            nc.sync.dma_start(
                                rs_in[ro0 : ro0 + ST, no * 512 : (no + 1) * 512],
                                ob[:, no * 512 : (no + 1) * 512],
                            )
                        if qc == N_QC - 1:
                            # mini-RS per 128-row subtile to shrink the tail
                            orow = qc * RROWS + si * (MR // NC)
                            if single:
                                nc.sync.dma_start(
                                    out_d[orow : orow + MR // NC, :],
                                    rs_in[0 : MR // NC, :],
                                )
                            else:
                                nc.gpsimd.collective_compute(
                                    "ReduceScatter",
                                    mybir.AluOpType.add,
                                    replica_groups=[list(range(NC))],
                                    ins=[rs_in.opt()],
                                    outs=[rs_out.opt()],
                                )
                                nc.sync.dma_start(
                                    out_d[orow : orow + MR // NC, :], rs_out[:]
                                )

                    if qc < N_QC - 1:
                        orow = qc * RROWS
                        if single:
                            nc.sync.dma_start(
                                out_d[orow : orow + RROWS, :], rs_in[0:RROWS, :]
                            )
                        else:
                            nc.gpsimd.collective_compute(
                                "ReduceScatter",
                                mybir.AluOpType.add,
                                replica_groups=[list(range(NC))],
                                ins=[rs_in.opt()],
                                outs=[rs_out.opt()],
                            )
                            nc.sync.dma_start(
                                out_d[orow : orow + RROWS, :], rs_out[:]
                            )

    nc.compile()
    return nc


def _get_nc():
    global _NC_CACHE
    if _NC_CACHE is None:
        _NC_CACHE = _build()
    return _NC_CACHE


def make_in_maps(inputs):
    X = np.asarray(inputs["hidden_states"], dtype=np.float32).reshape(S, HID)
    freqs = np.asarray(inputs["freqs_cis"], dtype=np.float32)
    Wq = np.asarray(inputs["Wq"], dtype=np.float32)
    Wk = np.asarray(inputs["Wk"], dtype=np.float32)
    Wv = np.asarray(inputs["Wv"], dtype=np.float32)
    Wo = np.asarray(inputs["Wo"], dtype=np.float32)
    qw = np.asarray(inputs["q_norm_w"], dtype=np.float32)
    kw = np.asarray(inputs["k_norm_w"], dtype=np.float32)

    bf = ml_dtypes.bfloat16
    # X^T load tiles: (L, ch, p, s) = X[L*XL+s, ch*128+p]
    xt = np.ascontiguousarray(
        X.reshape(N_XL, XL, HC, 128).transpose(0, 2, 3, 1).astype(bf)
    )
    cos, sin = freqs[0], freqs[1]  # [S, D]
    cwq = np.ascontiguousarray((cos * qw[None, :]).reshape(N_ST, 128, D))
    swq = np.ascontiguousarray((sin * np.roll(qw, D // 2)[None, :]).reshape(N_ST, 128, D))
    cwk = np.ascontiguousarray((cos * kw[None, :]).reshape(N_ST, 128, D))
    swk = np.ascontiguousarray((sin * np.roll(kw, D // 2)[None, :]).reshape(N_ST, 128, D))

    in_maps = []
    for c in range(NC):
        wq_c = Wq[c * DQ : (c + 1) * DQ, :]  # [DQ, HID]
        wq_t = np.ascontiguousarray(wq_c.T.reshape(HC, 128, DQ).astype(bf))
        wk_c = Wk[c * D : (c + 1) * D, :]
        wv_c = Wv[c * D : (c + 1) * D, :]
        wkv_t = np.ascontiguousarray(
            np.concatenate([wk_c.T, wv_c.T], axis=1).reshape(HC, 128, 2 * D).astype(bf)
        )
        wo_c = Wo[:, c * DQ : (c + 1) * DQ]  # [HID, DQ]
        wo_t = np.ascontiguousarray(wo_c.T.reshape(G, 128, HID).astype(bf))
        in_maps.append(
            {
                "xt": xt,
                "wq": wq_t,
                "wkv": wkv_t,
                "wo": wo_t,
                "cwq": cwq,
                "swq": swq,
                "cwk": cwk,
                "swk": swk,
            }
        )
    return in_maps


def assemble(outs):
    # outs[c] is [S//NC, HID] bf16.
    # Chunks qc=0..2: full-chunk RS over rows [512qc, +512); core c receives
    #   global rows [512qc + 64c, +64), stored at core-local rows [64qc, +64).
    # Chunk 3: four 128-row mini-RS over rows [1536 + 128i, +128); core c
    #   receives global rows [1536 + 128i + 16c, +16), stored locally at
    #   [192 + 16i, +16).
    y = np.empty((S, HID), dtype=np.float32)
    rows = QC // NC  # 64
    mrows = MR // NC  # 16
    for qc in range(N_QC - 1):
        for c in range(NC):
            g0 = QC * qc + rows * c
            l0 = rows * qc
            y[g0 : g0 + rows, :] = outs[c][l0 : l0 + rows, :].astype(np.float32)
    for i in range(QC // MR):
        for c in range(NC):
            g0 = QC * (N_QC - 1) + MR * i + mrows * c
            l0 = rows * (N_QC - 1) + mrows * i
            y[g0 : g0 + mrows, :] = outs[c][l0 : l0 + mrows, :].astype(np.float32)
    return y.reshape(B, S, HID)


def kernel(**inputs) -> np.ndarray:
    nc = _get_nc()
    in_maps = make_in_maps(inputs)
    res = bass_utils.run_bass_kernel_spmd(nc, in_maps, core_ids=list(range(NC)))
    return assemble([r["out"] for r in res.results])


# revision 11
# speedup vs baseline: 1.0389x; 1.0389x over previous
# GQA attention layer (B=1, S=2048, HID=2560, H=32, HKV=8, D=128) on 8 TRN2
# NeuronCores. Tensor-parallel over kv-head groups: core c owns kv head c and
# its 4 query heads (Wq/Wk/Wv row shards, Wo column shard). The o_proj
# partials are combined with an on-device ReduceScatter over the sequence
# axis; the host reassembles the sequence-sharded outputs.
#
# Per-core dataflow (all matmuls bf16 -> fp32 PSUM):
#   1. QKV projection from X^T tiles (s-major output layout), per-head
#      RMSNorm + RoPE on DVE, PE-transpose of Q/K into [d, s] layout.
#   2. Scores are computed transposed (S^T[k, q] = K Q^T) so that the
#      P^T @ V matmul needs no transpose of the 16.8M-element prob matrix.
#      exp() on the scalar engine in 2-PSUM-bank batches (no max
#      subtraction: |scores| is bounded). Scores and PV matmuls are
#      software-pipelined so the PE never waits on the exp stream.
#      Softmax denominators: pairwise kt-tree accumulation of the exp
#      tiles on DVE, then a single all-ones [128,128] stationary matmul
#      per unit broadcasts the partition-sums to every output row; one
#      DVE reciprocal + multiply normalizes the PV output.
#   3. o_proj per 512-row chunk (DVE evictions only - the scalar engine
#      stays exp-only in phase 2, avoiding ACT table reloads). The first
#      3 chunks ReduceScatter at 512 rows; the last chunk scatters per
#      128-row subtile so the final RS tail is ~4x shorter.
import sys

if "/opt/trn_rl_repo" not in sys.path:
    sys.path.insert(0, "/opt/trn_rl_repo")

import numpy as np
import ml_dtypes

import concourse.bacc as bacc
import concourse.mybir as mybir
import concourse.tile as tile
from concourse import bass_utils, masks

BF16 = mybir.dt.bfloat16
F32 = mybir.dt.float32

B, S, HID = 1, 2048, 2560
H, HKV, D = 32, 8, 128
G = H // HKV  # q heads per kv head (= per core)
NC = 8  # cores
DQ = G * D  # per-core q width (512)
EPS = 1e-6
SCALE = 1.0 / float(np.sqrt(D))

ST = 128          # s positions per compute tile
N_ST = S // ST    # 16
HC = HID // 128   # 20 contraction chunks
XL = 256          # s positions per X^T DMA load tile
N_XL = S // XL    # 8
QC = 512          # q positions per attention unit
N_QC = S // QC    # 4
N_KT = S // 128   # 16 k tiles per attention unit
NP = N_KT // 2    # 8 score/exp pairs per unit
NO = HID // 512   # 5 o_proj free-dim chunks
MR = 128          # rows per mini-ReduceScatter (last chunk)

_NC_CACHE = None


def _build(reps: int = 1, single: bool = False):
    nc = bacc.Bacc(
        "TRN2", target_bir_lowering=False, debug=False,
        num_devices=(1 if single else NC),
    )

    xt_d = nc.dram_tensor("xt", [N_XL, HC, 128, XL], BF16, kind="ExternalInput").ap()
    wq_d = nc.dram_tensor("wq", [HC, 128, DQ], BF16, kind="ExternalInput").ap()
    wkv_d = nc.dram_tensor("wkv", [HC, 128, 2 * D], BF16, kind="ExternalInput").ap()
    wo_d = nc.dram_tensor("wo", [G, 128, HID], BF16, kind="ExternalInput").ap()
    cwq_d = nc.dram_tensor("cwq", [N_ST, 128, D], F32, kind="ExternalInput").ap()
    swq_d = nc.dram_tensor("swq", [N_ST, 128, D], F32, kind="ExternalInput").ap()
    cwk_d = nc.dram_tensor("cwk", [N_ST, 128, D], F32, kind="ExternalInput").ap()
    swk_d = nc.dram_tensor("swk", [N_ST, 128, D], F32, kind="ExternalInput").ap()
    out_d = nc.dram_tensor("out", [S // NC, HID], BF16, kind="ExternalOutput").ap()

    with tile.TileContext(nc) as tc:
        with (
            tc.tile_pool(name="const", bufs=1) as cpool,
            tc.tile_pool(name="xt", bufs=2) as xt_pool,
            tc.tile_pool(name="cs", bufs=8) as cs_pool,
            tc.tile_pool(name="qw", bufs=5) as qw_pool,
            tc.tile_pool(name="kw", bufs=6) as kw_pool,
            tc.tile_pool(name="ro", bufs=2) as ro_pool,
            tc.tile_pool(name="sm", bufs=4) as sm_pool,
            tc.tile_pool(name="ep", bufs=3) as ep_pool,
            tc.tile_pool(name="tr", bufs=2) as tr_pool,
            tc.tile_pool(name="ot", bufs=8) as ot_pool,
            tc.tile_pool(name="ob", bufs=2) as ob_pool,
            tc.tile_pool(name="psA", bufs=2, space="PSUM") as psA,
            tc.tile_pool(name="psB", bufs=2, space="PSUM") as psB,
            tc.tile_pool(name="psY", bufs=1, space="PSUM") as psY,
            tc.tile_pool(name="psC", bufs=1, space="PSUM") as psC,
            tc.tile_pool(name="dram", bufs=1, space="DRAM") as dram,
        ):
            for _rep in range(reps):
                # ---- resident constants / weights ----
                ident = cpool.tile([128, 128], BF16, tag="ident")
                masks.make_identity(nc, ident[:])
                # all-ones stationary: one sums matmul per unit yields the
                # softmax denominator replicated across all 128 partitions
                ones_k = cpool.tile([128, 128], BF16, tag="ones_k")
                nc.vector.memset(ones_k[:], 1.0)

                # interleave per-chunk weight + xt[0] chunk loads so the
                # first st=0 matmuls start as soon as chunk 0 lands
                xt_t = xt_pool.tile([128, HC, XL], BF16, tag="xt")
                wq_t = []
                wkv_t = []
                for ch in range(HC):
                    w1 = cpool.tile([128, DQ], BF16, tag=f"wq{ch}")
                    nc.sync.dma_start(w1[:], wq_d[ch])
                    wq_t.append(w1)
                    w2 = cpool.tile([128, 2 * D], BF16, tag=f"wkv{ch}")
                    nc.sync.dma_start(w2[:], wkv_d[ch])
                    wkv_t.append(w2)
                    nc.sync.dma_start(xt_t[:, ch, :], xt_d[0, ch])
                xt_next = xt_pool.tile([128, HC, XL], BF16, tag="xt")
                nc.sync.dma_start(xt_next[:], xt_d[1].rearrange("c p s -> p c s"))

                qt_sb = cpool.tile([128, G, S], BF16, tag="qt")   # Q^T  [d, h, s]
                kt_sb = cpool.tile([128, S], BF16, tag="kt")      # K^T  [d, s]
                v_sb = cpool.tile([128, N_KT, D], BF16, tag="v")  # V    [s%128, kt, d]

                # ================= phase 1: QKV + norm + rope + transpose ======
                for st in range(N_ST):
                    if st % (XL // ST) == 0 and st > 0:
                        if st // (XL // ST) == 1:
                            xt_t = xt_next
                        else:
                            xt_t = xt_pool.tile([128, HC, XL], BF16, tag="xt")
                            nc.sync.dma_start(
                                xt_t[:],
                                xt_d[st // (XL // ST)].rearrange("c p s -> p c s"),
                            )
                    soff = (st % (XL // ST)) * ST

                    cwq_t = cs_pool.tile([128, D], F32, tag="cs")
                    nc.sync.dma_start(cwq_t[:], cwq_d[st])
                    swq_t = cs_pool.tile([128, D], F32, tag="cs")
                    nc.sync.dma_start(swq_t[:], swq_d[st])
                    cwk_t = cs_pool.tile([128, D], F32, tag="cs")
                    nc.sync.dma_start(cwk_t[:], cwk_d[st])
                    swk_t = cs_pool.tile([128, D], F32, tag="cs")
                    nc.sync.dma_start(swk_t[:], swk_d[st])

                    # q in bank 0, kv in the low half of bank 1
                    qkv_ps = psA.tile([128, 2, DQ], F32, tag="a")
                    for ch in range(HC):
                        lhs = xt_t[:, ch, soff : soff + ST]
                        nc.tensor.matmul(
                            qkv_ps[:, 0, :], lhs, wq_t[ch][:],
                            start=(ch == 0), stop=(ch == HC - 1),
                        )
                        nc.tensor.matmul(
                            qkv_ps[:, 1, 0 : 2 * D], lhs, wkv_t[ch][:],
                            start=(ch == 0), stop=(ch == HC - 1),
                        )

                    # evictions (scalar engine; phase 1 is Copy-table only)
                    q_sb = qw_pool.tile([128, DQ], F32, tag="qw")
                    nc.scalar.copy(q_sb[:], qkv_ps[:, 0, :])
                    k_sb = kw_pool.tile([128, D], F32, tag="kw")
                    nc.scalar.copy(k_sb[:], qkv_ps[:, 1, 0:D])
                    nc.scalar.copy(v_sb[:, st, :], qkv_ps[:, 1, D : 2 * D])

                    # ---- RMSNorm (per head) ----
                    sq = qw_pool.tile([128, DQ], F32, tag="qw")
                    nc.vector.tensor_mul(sq[:], q_sb[:], q_sb[:])
                    ssq = sm_pool.tile([128, G + 1], F32, tag="sm")
                    nc.vector.tensor_reduce(
                        ssq[:, 0:G], sq[:].rearrange("p (h d) -> p h d", d=D),
                        axis=mybir.AxisListType.X, op=mybir.AluOpType.add,
                    )
                    ksq = kw_pool.tile([128, D], F32, tag="kw")
                    nc.vector.tensor_mul(ksq[:], k_sb[:], k_sb[:])
                    nc.vector.tensor_reduce(
                        ssq[:, G : G + 1], ksq[:].unsqueeze(1),
                        axis=mybir.AxisListType.X, op=mybir.AluOpType.add,
                    )
                    var = sm_pool.tile([128, G + 1], F32, tag="sm")
                    nc.vector.tensor_scalar(
                        var[:], ssq[:], 1.0 / D, EPS,
                        op0=mybir.AluOpType.mult, op1=mybir.AluOpType.add,
                    )
                    rt = sm_pool.tile([128, G + 1], F32, tag="sm")
                    nc.scalar.sqrt(rt[:], var[:])
                    rq = sm_pool.tile([128, G + 1], F32, tag="sm")
                    nc.vector.reciprocal(rq[:], rt[:])
                    rk = rq

                    # ---- normalize + rope (DVE) ----
                    qn = qw_pool.tile([128, DQ], F32, tag="qw")
                    qn3 = qn[:].rearrange("p (h d) -> p h d", d=D)
                    nc.vector.tensor_tensor(
                        qn3, q_sb[:].rearrange("p (h d) -> p h d", d=D),
                        rq[:, 0:G].unsqueeze(2).to_broadcast([128, G, D]),
                        op=mybir.AluOpType.mult,
                    )
                    t1 = qw_pool.tile([128, DQ], F32, tag="qw")
                    t13 = t1[:].rearrange("p (h d) -> p h d", d=D)
                    cwq3 = cwq_t[:].unsqueeze(1).to_broadcast([128, G, D])
                    swq3 = swq_t[:].unsqueeze(1).to_broadcast([128, G, D])
                    nc.vector.tensor_tensor(t13, qn3, cwq3, op=mybir.AluOpType.mult)
                    u = qw_pool.tile([128, DQ], F32, tag="qw")
                    u3 = u[:].rearrange("p (h d) -> p h d", d=D)
                    hd = D // 2
                    nc.vector.tensor_tensor(
                        u3[:, :, 0:hd], qn3[:, :, hd:D], swq3[:, :, 0:hd],
                        op=mybir.AluOpType.mult,
                    )
                    nc.vector.tensor_tensor(
                        u3[:, :, hd:D], qn3[:, :, 0:hd], swq3[:, :, hd:D],
                        op=mybir.AluOpType.mult,
                    )
                    qro = ro_pool.tile([128, DQ], BF16, tag="qro")
                    qro3 = qro[:].rearrange("p (h d) -> p h d", d=D)
                    nc.vector.tensor_sub(qro3[:, :, 0:hd], t13[:, :, 0:hd], u3[:, :, 0:hd])
                    nc.vector.tensor_add(qro3[:, :, hd:D], t13[:, :, hd:D], u3[:, :, hd:D])

                    kn = kw_pool.tile([128, D], F32, tag="kw")
                    nc.vector.tensor_tensor(
                        kn[:], k_sb[:],
                        rk[:, G : G + 1].to_broadcast([128, D]),
                        op=mybir.AluOpType.mult,
                    )
                    kt1 = kw_pool.tile([128, D], F32, tag="kw")
                    nc.vector.tensor_tensor(kt1[:], kn[:], cwk_t[:], op=mybir.AluOpType.mult)
                    ku = kw_pool.tile([128, D], F32, tag="kw")
                    nc.vector.tensor_tensor(
                        ku[:, 0:hd], kn[:, hd:D], swk_t[:, 0:hd], op=mybir.AluOpType.mult
                    )
                    nc.vector.tensor_tensor(
                        ku[:, hd:D], kn[:, 0:hd], swk_t[:, hd:D], op=mybir.AluOpType.mult
                    )
                    kro = ro_pool.tile([128, D], BF16, tag="kro")
                    nc.vector.tensor_sub(kro[:, 0:hd], kt1[:, 0:hd], ku[:, 0:hd])
                    nc.vector.tensor_add(kro[:, hd:D], kt1[:, hd:D], ku[:, hd:D])

                    # ---- transpose Q heads + K into [d, s] ----
                    # alternate the two 1-buf PSUM pools for 2-deep rotation
                    for h in range(G):
                        tpool = psC if h % 2 == 0 else psY
                        ttag = "c" if h % 2 == 0 else "y"
                        tp = tpool.tile([128, 128], BF16, tag=ttag, name="tp")
                        nc.tensor.transpose(tp[:], qro[:, h * D : (h + 1) * D], ident[:])
                        nc.scalar.copy(qt_sb[:, h, st * ST : (st + 1) * ST], tp[:])
                    tp = psC.tile([128, 128], BF16, tag="c")
                    nc.tensor.transpose(tp[:], kro[:], ident[:])
                    nc.scalar.copy(kt_sb[:, st * ST : (st + 1) * ST], tp[:])

                # ================= phase 2: attention + o_proj + RS ============
                # wo is first needed ~20us into phase 2; load it behind the
                # phase-1 traffic instead of ahead of it
                wo_sb = cpool.tile([128, G, HID], BF16, tag="wo")
                nc.sync.dma_start(wo_sb[:], wo_d.rearrange("c p n -> p c n"))

                # Slot pipeline over 16 attention units u = (qc, h):
                #   slot t: scores+exp+L1(u_t) interleaved with pv(u_{t-1}),
                #   then ones/recip/ot(u_{t-1}), then one o_proj 128-row
                #   si-block of chunk t//4 - 1 (chunk-lagged), tree L2-4 of
                #   u_t on gpsimd (latency-tolerant: result needed a full
                #   slot later). The exp stream (scalar) and the sums tree
                #   never gate the PE.
                NU = N_QC * G  # 16 units
                RROWS = QC // NC  # 64 output rows per core per RS chunk
                unit_state = [None] * NU  # (ep, pv_ps, tr1, colsum, rb)
                ot_tiles = {}
                rs_tiles = {}

                def emit_scores_pair(t, p):
                    ep, _, tr1 = unit_state[t][:3]
                    qsl = qt_sb[:, t % G, (t // G) * QC : (t // G + 1) * QC]
                    s2 = psA.tile([128, 2, QC], F32, tag="a")
                    nc.tensor.matmul(
                        s2[:, 0, :],
                        kt_sb[:, (2 * p) * 128 : (2 * p + 1) * 128],
                        qsl, start=True, stop=True,
                    )
                    nc.tensor.matmul(
                        s2[:, 1, :],
                        kt_sb[:, (2 * p + 1) * 128 : (2 * p + 2) * 128],
                        qsl, start=True, stop=True,
                    )
                    # one exp over both PSUM banks
                    nc.scalar.activation(
                        ep[:, 2 * p : 2 * p + 2, :], s2[:],
                        mybir.ActivationFunctionType.Exp, scale=SCALE,
                    )
                    # kt-tree level 1 on DVE
                    tr = tr_pool.tile([128, QC], BF16, tag="t1", bufs=9)
                    nc.vector.tensor_add(tr[:], ep[:, 2 * p, :], ep[:, 2 * p + 1, :])
                    tr1.append(tr)

                def emit_oproj_block(b):
                    qc_o, si = b // 4, b % 4
                    if si == 0:
                        rs_in = dram.tile([QC, HID], BF16, tag=f"rsin{qc_o}", name="rs_in")
                        rs_out = dram.tile([RROWS, HID], BF16, tag=f"rsout{qc_o}", name="rs_out")
                        rs_tiles[qc_o] = (rs_in, rs_out)
                    rs_in, rs_out = rs_tiles[qc_o]
                    ob = ob_pool.tile([128, HID], BF16, tag="ob")
                    for no in range(NO):
                        y_ps = psY.tile([128, 512], F32, tag="y")
                        for h in range(G):
                            nc.tensor.matmul(
                                y_ps[:],
                                ot_tiles[qc_o * G + h][:, si * ST : (si + 1) * ST],
                                wo_sb[:, h, no * 512 : (no + 1) * 512],
                                start=(h == 0), stop=(h == G - 1),
                            )
                        nc.vector.tensor_copy(
                            ob[:, no * 512 : (no + 1) * 512], y_ps[:]
                        )
                        nc.sync.dma_start(
                            rs_in[si * ST : (si + 1) * ST, no * 512 : (no + 1) * 512],
                            ob[:, no * 512 : (no + 1) * 512],
                        )
                    if si == 3:
                        orow = qc_o * RROWS
                        if single:
                            nc.sync.dma_start(
                                out_d[orow : orow + RROWS, :], rs_in[0:RROWS, :]
                            )
                        else:
                            nc.gpsimd.collective_compute(
                                "ReduceScatter",
                                mybir.AluOpType.add,
                                replica_groups=[list(range(NC))],
                                ins=[rs_in.opt()],
                                outs=[rs_out.opt()],
                            )
                            nc.sync.dma_start(
                                out_d[orow : orow + RROWS, :], rs_out[:]
                            )

                for t in range(NU + 1):
                    if t < NU:
                        ep = ep_pool.tile([128, N_KT, QC], BF16, tag="ep")
                        pv_ps = psB.tile([128, QC], F32, tag="b")
                        unit_state[t] = [ep, pv_ps, []]
                    # ---- interleaved scores(u_t) / pv(u_{t-1}) ----
                    for p in range(NP):
                        if t < NU:
                            emit_scores_pair(t, p)
                        if t >= 1:
                            epm, pvm = unit_state[t - 1][:2]
                            nc.tensor.matmul(
                                pvm[:], v_sb[:, 2 * p, :], epm[:, 2 * p, :],
                                start=(p == 0), stop=False,
                            )
                            nc.tensor.matmul(
                                pvm[:], v_sb[:, 2 * p + 1, :], epm[:, 2 * p + 1, :],
                                start=False, stop=(p == NP - 1),
                            )
                    # ---- finish unit u_{t-1}: ones-mm, recip, ot ----
                    if t >= 1:
                        epm, pvm, _, colsum = unit_state[t - 1]
                        sums_ps = psC.tile([128, QC], F32, tag="c")
                        nc.tensor.matmul(
                            sums_ps[:], ones_k[:], colsum[:], start=True, stop=True
                        )
                        rb = sm_pool.tile([128, QC], F32, tag="rb", bufs=2)
                        nc.vector.reciprocal_approx_fast(rb[:], sums_ps[:])
                        ot = ot_pool.tile([128, QC], BF16, tag="ot")
                        nc.vector.tensor_tensor(
                            ot[:], pvm[:], rb[:], op=mybir.AluOpType.mult
                        )
                        ot_tiles[t - 1] = ot
                    # ---- one chunk-lagged o_proj block per slot ----
                    if t < NU:
                        if 4 <= t:
                            emit_oproj_block(t - 4)
                    else:
                        for b in range(NU - 4, NU):
                            emit_oproj_block(b)
                    # ---- kt-tree levels 2..4 of u_t on gpsimd ----
                    if t < NU:
                        tr1 = unit_state[t][2]
                        tr2 = []
                        for j in range(4):
                            tr = tr_pool.tile([128, QC], BF16, tag="t2", bufs=5)
                            nc.gpsimd.tensor_add(tr[:], tr1[2 * j][:], tr1[2 * j + 1][:])
                            tr2.append(tr)
                        tr3 = []
                        for j in range(2):
                            tr = tr_pool.tile([128, QC], BF16, tag="t3", bufs=4)
                            nc.gpsimd.tensor_add(tr[:], tr2[2 * j][:], tr2[2 * j + 1][:])
                            tr3.append(tr)
                        colsum = tr_pool.tile([128, QC], BF16, tag="t4", bufs=4)
                        nc.gpsimd.tensor_add(colsum[:], tr3[0][:], tr3[1][:])
                        unit_state[t].append(colsum)

    nc.compile()
    return nc


def _get_nc():
    global _NC_CACHE
    if _NC_CACHE is None:
        _NC_CACHE = _build()
    return _NC_CACHE


def make_in_maps(inputs):
    X = np.asarray(inputs["hidden_states"], dtype=np.float32).reshape(S, HID)
    freqs = np.asarray(inputs["freqs_cis"], dtype=np.float32)
    Wq = np.asarray(inputs["Wq"], dtype=np.float32)
    Wk = np.asarray(inputs["Wk"], dtype=np.float32)
    Wv = np.asarray(inputs["Wv"], dtype=np.float32)
    Wo = np.asarray(inputs["Wo"], dtype=np.float32)
    qw = np.asarray(inputs["q_norm_w"], dtype=np.float32)
    kw = np.asarray(inputs["k_norm_w"], dtype=np.float32)

    bf = ml_dtypes.bfloat16
    # X^T load tiles: (L, ch, p, s) = X[L*XL+s, ch*128+p]
    xt = np.ascontiguousarray(
        X.reshape(N_XL, XL, HC, 128).transpose(0, 2, 3, 1).astype(bf)
    )
    cos, sin = freqs[0], freqs[1]  # [S, D]
    cwq = np.ascontiguousarray((cos * qw[None, :]).reshape(N_ST, 128, D))
    swq = np.ascontiguousarray((sin * np.roll(qw, D // 2)[None, :]).reshape(N_ST, 128, D))
    cwk = np.ascontiguousarray((cos * kw[None, :]).reshape(N_ST, 128, D))
    swk = np.ascontiguousarray((sin * np.roll(kw, D // 2)[None, :]).reshape(N_ST, 128, D))

    in_maps = []
    for c in range(NC):
        wq_c = Wq[c * DQ : (c + 1) * DQ, :]  # [DQ, HID]
        wq_t = np.ascontiguousarray(wq_c.T.reshape(HC, 128, DQ).astype(bf))
        wk_c = Wk[c * D : (c + 1) * D, :]
        wv_c = Wv[c * D : (c + 1) * D, :]
        wkv_t = np.ascontiguousarray(
            np.concatenate([wk_c.T, wv_c.T], axis=1).reshape(HC, 128, 2 * D).astype(bf)
        )
        wo_c = Wo[:, c * DQ : (c + 1) * DQ]  # [HID, DQ]
        wo_t = np.ascontiguousarray(wo_c.T.reshape(G, 128, HID).astype(bf))
        in_maps.append(
            {
                "xt": xt,
                "wq": wq_t,
                "wkv": wkv_t,
                "wo": wo_t,
                "cwq": cwq,
                "swq": swq,
                "cwk": cwk,
                "swk": swk,
            }
        )
    return in_maps


def assemble(outs):
    # outs[c] is [S//NC, HID] bf16. RS chunk qc covers global rows
    # [512*qc, +512); core c receives rows [64*c, 64*c+64) of it,
    # stored at core-local rows [64*qc, +64).
    y = np.empty((S, HID), dtype=np.float32)
    rows = QC // NC  # 64
    for qc in range(N_QC):
        for c in range(NC):
            g0 = QC * qc + rows * c
            l0 = rows * qc
            y[g0 : g0 + rows, :] = outs[c][l0 : l0 + rows, :].astype(np.float32)
    return y.reshape(B, S, HID)


def kernel(**inputs) -> np.ndarray:
    nc = _get_nc()
    in_maps = make_in_maps(inputs)
    res = bass_utils.run_bass_kernel_spmd(nc, in_maps, core_ids=list(range(NC)))
    return assemble([r["out"] for r in res.results])


# revision 16
# speedup vs baseline: 1.0530x; 1.0136x over previous
# GQA attention layer (B=1, S=2048, HID=2560, H=32, HKV=8, D=128) on 8 TRN2
# NeuronCores. Tensor-parallel over kv-head groups: core c owns kv head c and
# its 4 query heads (Wq/Wk/Wv row shards, Wo column shard). The o_proj
# partials are combined with an on-device ReduceScatter over the sequence
# axis; the host reassembles the sequence-sharded outputs.
#
# Per-core dataflow (all matmuls bf16 -> fp32 PSUM):
#   1. QKV projection from X^T tiles (s-major output layout), per-head
#      RMSNorm + RoPE on DVE, PE-transpose of Q/K into [d, s] layout.
#   2. Scores are computed transposed (S^T[k, q] = K Q^T) so that the
#      P^T @ V matmul needs no transpose of the 16.8M-element prob matrix.
#      exp() on the scalar engine in 2-PSUM-bank batches (no max
#      subtraction: |scores| is bounded). Scores and PV matmuls are
#      software-pipelined so the PE never waits on the exp stream.
#      Softmax denominators: pairwise kt-tree accumulation of the exp
#      tiles on DVE, then a single all-ones [128,128] stationary matmul
#      per unit broadcasts the partition-sums to every output row; one
#      DVE reciprocal + multiply normalizes the PV output.
#   3. o_proj per 512-row chunk (DVE evictions only - the scalar engine
#      stays exp-only in phase 2, avoiding ACT table reloads). The first
#      3 chunks ReduceScatter at 512 rows; the last chunk scatters per
#      128-row subtile so the final RS tail is ~4x shorter.
import sys

if "/opt/trn_rl_repo" not in sys.path:
    sys.path.insert(0, "/opt/trn_rl_repo")

import numpy as np
import ml_dtypes

import concourse.bacc as bacc
import concourse.mybir as mybir
import concourse.tile as tile
from concourse import bass_utils, masks

BF16 = mybir.dt.bfloat16
F32 = mybir.dt.float32

B, S, HID = 1, 2048, 2560
H, HKV, D = 32, 8, 128
G = H // HKV  # q heads per kv head (= per core)
NC = 8  # cores
DQ = G * D  # per-core q width (512)
EPS = 1e-6
SCALE = 1.0 / float(np.sqrt(D))

ST = 128          # s positions per compute tile
N_ST = S // ST    # 16
HC = HID // 128   # 20 contraction chunks
XL = 256          # s positions per X^T DMA load tile
N_XL = S // XL    # 8
QC = 512          # q positions per attention unit
N_QC = S // QC    # 4
N_KT = S // 128   # 16 k tiles per attention unit
NP = N_KT // 2    # 8 score/exp pairs per unit
NO = HID // 512   # 5 o_proj free-dim chunks
MR = 128          # rows per mini-ReduceScatter (last chunk)

_NC_CACHE = None


def _build(reps: int = 1, single: bool = False):
    nc = bacc.Bacc(
        "TRN2", target_bir_lowering=False, debug=False,
        num_devices=(1 if single else NC),
    )

    xt_d = nc.dram_tensor("xt", [N_XL, HC, 128, XL], BF16, kind="ExternalInput").ap()
    wq_d = nc.dram_tensor("wq", [HC, 128, DQ], BF16, kind="ExternalInput").ap()
    wkv_d = nc.dram_tensor("wkv", [HC, 128, 2 * D], BF16, kind="ExternalInput").ap()
    wo_d = nc.dram_tensor("wo", [G, 128, HID], BF16, kind="ExternalInput").ap()
    cwq_d = nc.dram_tensor("cwq", [N_ST, 128, D], F32, kind="ExternalInput").ap()
    swq_d = nc.dram_tensor("swq", [N_ST, 128, D], F32, kind="ExternalInput").ap()
    cwk_d = nc.dram_tensor("cwk", [N_ST, 128, D], F32, kind="ExternalInput").ap()
    swk_d = nc.dram_tensor("swk", [N_ST, 128, D], F32, kind="ExternalInput").ap()
    out_d = nc.dram_tensor("out", [S // NC, HID], BF16, kind="ExternalOutput").ap()

    with tile.TileContext(nc) as tc:
        with (
            tc.tile_pool(name="const", bufs=1) as cpool,
            tc.tile_pool(name="xt", bufs=2) as xt_pool,
            tc.tile_pool(name="cs", bufs=8) as cs_pool,
            tc.tile_pool(name="qw", bufs=5) as qw_pool,
            tc.tile_pool(name="kw", bufs=6) as kw_pool,
            tc.tile_pool(name="ro", bufs=2) as ro_pool,
            tc.tile_pool(name="sm", bufs=4) as sm_pool,
            tc.tile_pool(name="ep", bufs=2) as ep_pool,
            tc.tile_pool(name="tr", bufs=2) as tr_pool,
            tc.tile_pool(name="ot", bufs=8) as ot_pool,
            tc.tile_pool(name="ob", bufs=3) as ob_pool,
            tc.tile_pool(name="psA", bufs=4, space="PSUM") as psA,
            tc.tile_pool(name="psB", bufs=2, space="PSUM") as psB,
            tc.tile_pool(name="psY", bufs=2, space="PSUM") as psY,
            tc.tile_pool(name="dram", bufs=1, space="DRAM") as dram,
        ):
            for _rep in range(reps):
                # ---- resident constants / weights ----
                ident = cpool.tile([128, 128], BF16, tag="ident")
                masks.make_identity(nc, ident[:])
                # all-ones stationary: one sums matmul per unit yields the
                # softmax denominator replicated across all 128 partitions
                ones_k = cpool.tile([128, 128], BF16, tag="ones_k")
                nc.vector.memset(ones_k[:], 1.0)

                # interleave per-chunk weight + xt[0] chunk loads so the
                # first st=0 matmuls start as soon as chunk 0 lands
                xt_t = xt_pool.tile([128, HC, XL], BF16, tag="xt")
                wq_t = []
                wkv_t = []
                for ch in range(HC):
                    w1 = cpool.tile([128, DQ], BF16, tag=f"wq{ch}")
                    nc.sync.dma_start(w1[:], wq_d[ch])
                    wq_t.append(w1)
                    w2 = cpool.tile([128, 2 * D], BF16, tag=f"wkv{ch}")
                    nc.sync.dma_start(w2[:], wkv_d[ch])
                    wkv_t.append(w2)
                    nc.sync.dma_start(xt_t[:, ch, :], xt_d[0, ch])
                xt_next = xt_pool.tile([128, HC, XL], BF16, tag="xt")
                nc.sync.dma_start(xt_next[:], xt_d[1].rearrange("c p s -> p c s"))

                qt_sb = cpool.tile([128, G, S], BF16, tag="qt")   # Q^T  [d, h, s]
                kt_sb = cpool.tile([128, S], BF16, tag="kt")      # K^T  [d, s]
                v_sb = cpool.tile([128, N_KT, D], BF16, tag="v")  # V    [s%128, kt, d]

                # ================= phase 1: QKV + norm + rope + transpose ======
                for st in range(N_ST):
                    if st % (XL // ST) == 0 and st > 0:
                        if st // (XL // ST) == 1:
                            xt_t = xt_next
                        else:
                            xt_t = xt_pool.tile([128, HC, XL], BF16, tag="xt")
                            nc.sync.dma_start(
                                xt_t[:],
                                xt_d[st // (XL // ST)].rearrange("c p s -> p c s"),
                            )
                    soff = (st % (XL // ST)) * ST

                    cwq_t = cs_pool.tile([128, D], F32, tag="cs")
                    nc.sync.dma_start(cwq_t[:], cwq_d[st])
                    swq_t = cs_pool.tile([128, D], F32, tag="cs")
                    nc.sync.dma_start(swq_t[:], swq_d[st])
                    cwk_t = cs_pool.tile([128, D], F32, tag="cs")
                    nc.sync.dma_start(cwk_t[:], cwk_d[st])
                    swk_t = cs_pool.tile([128, D], F32, tag="cs")
                    nc.sync.dma_start(swk_t[:], swk_d[st])

                    q_ps = psA.tile([128, DQ], F32, tag="s")
                    kv_ps = psA.tile([128, DQ], F32, tag="s")
                    for ch in range(HC):
                        lhs = xt_t[:, ch, soff : soff + ST]
                        nc.tensor.matmul(
                            q_ps[:], lhs, wq_t[ch][:],
                            start=(ch == 0), stop=(ch == HC - 1),
                        )
                        nc.tensor.matmul(
                            kv_ps[:, 0 : 2 * D], lhs, wkv_t[ch][:],
                            start=(ch == 0), stop=(ch == HC - 1),
                        )

                    # evictions (scalar engine; phase 1 is Copy-table only)
                    q_sb = qw_pool.tile([128, DQ], F32, tag="qw")
                    nc.scalar.copy(q_sb[:], q_ps[:])
                    k_sb = kw_pool.tile([128, D], F32, tag="kw")
                    nc.scalar.copy(k_sb[:], kv_ps[:, 0:D])
                    nc.scalar.copy(v_sb[:, st, :], kv_ps[:, D : 2 * D])

                    # ---- RMSNorm (per head) ----
                    sq = qw_pool.tile([128, DQ], F32, tag="qw")
                    nc.vector.tensor_mul(sq[:], q_sb[:], q_sb[:])
                    ssq = sm_pool.tile([128, G + 1], F32, tag="sm")
                    nc.vector.tensor_reduce(
                        ssq[:, 0:G], sq[:].rearrange("p (h d) -> p h d", d=D),
                        axis=mybir.AxisListType.X, op=mybir.AluOpType.add,
                    )
                    ksq = kw_pool.tile([128, D], F32, tag="kw")
                    nc.vector.tensor_mul(ksq[:], k_sb[:], k_sb[:])
                    nc.vector.tensor_reduce(
                        ssq[:, G : G + 1], ksq[:].unsqueeze(1),
                        axis=mybir.AxisListType.X, op=mybir.AluOpType.add,
                    )
                    var = sm_pool.tile([128, G + 1], F32, tag="sm")
                    nc.vector.tensor_scalar(
                        var[:], ssq[:], 1.0 / D, EPS,
                        op0=mybir.AluOpType.mult, op1=mybir.AluOpType.add,
                    )
                    rt = sm_pool.tile([128, G + 1], F32, tag="sm")
                    nc.scalar.sqrt(rt[:], var[:])
                    rq = sm_pool.tile([128, G + 1], F32, tag="sm")
                    nc.vector.reciprocal(rq[:], rt[:])
                    rk = rq

                    # ---- normalize + rope (DVE) ----
                    qn = qw_pool.tile([128, DQ], F32, tag="qw")
                    qn3 = qn[:].rearrange("p (h d) -> p h d", d=D)
                    nc.vector.tensor_tensor(
                        qn3, q_sb[:].rearrange("p (h d) -> p h d", d=D),
                        rq[:, 0:G].unsqueeze(2).to_broadcast([128, G, D]),
                        op=mybir.AluOpType.mult,
                    )
                    t1 = qw_pool.tile([128, DQ], F32, tag="qw")
                    t13 = t1[:].rearrange("p (h d) -> p h d", d=D)
                    cwq3 = cwq_t[:].unsqueeze(1).to_broadcast([128, G, D])
                    swq3 = swq_t[:].unsqueeze(1).to_broadcast([128, G, D])
                    nc.vector.tensor_tensor(t13, qn3, cwq3, op=mybir.AluOpType.mult)
                    u = qw_pool.tile([128, DQ], F32, tag="qw")
                    u3 = u[:].rearrange("p (h d) -> p h d", d=D)
                    hd = D // 2
                    nc.vector.tensor_tensor(
                        u3[:, :, 0:hd], qn3[:, :, hd:D], swq3[:, :, 0:hd],
                        op=mybir.AluOpType.mult,
                    )
                    nc.vector.tensor_tensor(
                        u3[:, :, hd:D], qn3[:, :, 0:hd], swq3[:, :, hd:D],
                        op=mybir.AluOpType.mult,
                    )
                    qro = ro_pool.tile([128, DQ], BF16, tag="qro")
                    qro3 = qro[:].rearrange("p (h d) -> p h d", d=D)
                    nc.vector.tensor_sub(qro3[:, :, 0:hd], t13[:, :, 0:hd], u3[:, :, 0:hd])
                    nc.vector.tensor_add(qro3[:, :, hd:D], t13[:, :, hd:D], u3[:, :, hd:D])

                    kn = kw_pool.tile([128, D], F32, tag="kw")
                    nc.vector.tensor_tensor(
                        kn[:], k_sb[:],
                        rk[:, G : G + 1].to_broadcast([128, D]),
                        op=mybir.AluOpType.mult,
                    )
                    kt1 = kw_pool.tile([128, D], F32, tag="kw")
                    nc.vector.tensor_tensor(kt1[:], kn[:], cwk_t[:], op=mybir.AluOpType.mult)
                    ku = kw_pool.tile([128, D], F32, tag="kw")
                    nc.vector.tensor_tensor(
                        ku[:, 0:hd], kn[:, hd:D], swk_t[:, 0:hd], op=mybir.AluOpType.mult
                    )
                    nc.vector.tensor_tensor(
                        ku[:, hd:D], kn[:, 0:hd], swk_t[:, hd:D], op=mybir.AluOpType.mult
                    )
                    kro = ro_pool.tile([128, D], BF16, tag="kro")
                    nc.vector.tensor_sub(kro[:, 0:hd], kt1[:, 0:hd], ku[:, 0:hd])
                    nc.vector.tensor_add(kro[:, hd:D], kt1[:, hd:D], ku[:, hd:D])

                    # ---- transpose Q heads + K into [d, s] ----
                    for h in range(G):
                        tp = psY.tile([128, 128], BF16, tag="y", name="tp")
                        nc.tensor.transpose(tp[:], qro[:, h * D : (h + 1) * D], ident[:])
                        nc.scalar.copy(qt_sb[:, h, st * ST : (st + 1) * ST], tp[:])
                    tp = psY.tile([128, 128], BF16, tag="y")
                    nc.tensor.transpose(tp[:], kro[:], ident[:])
                    nc.scalar.copy(kt_sb[:, st * ST : (st + 1) * ST], tp[:])

                # ================= phase 2: attention + o_proj + RS ============
                # wo is first needed ~20us into phase 2; load it behind the
                # phase-1 traffic instead of ahead of it
                wo_sb = cpool.tile([128, G, HID], BF16, tag="wo")
                nc.sync.dma_start(wo_sb[:], wo_d.rearrange("c p n -> p c n"))

                # Slot pipeline over 16 attention units u = (qc, h):
                #   slot t: per-kt interleave of scores+exp+L1(u_t), pv(u_{t-1}),
                #   and the y-groups of one o_proj 128-row si-block (chunk-lagged
                #   by 1 slot past the chunk boundary so the last ot is ready).
                #   ones/recip/ot(u_{t-1}) at slot end; tree L2-4 of u_t on
                #   gpsimd (latency-tolerant: result needed a full slot later).
                #   The exp stream (scalar) and the sums tree never gate the PE;
                #   ob bufs=3 + psY bufs=2 ride out DMA outages while RS wire
                #   traffic hogs the queues.
                NU = N_QC * G  # 16 units
                RROWS = QC // NC  # 64 output rows per core per RS chunk
                unit_state = [None] * NU  # [ep, pv_ps, tr1, colsum]
                ot_tiles = {}
                rs_tiles = {}

                def emit_scores_kt(t, kt):
                    ep, _, tr1 = unit_state[t][:3]
                    qsl = qt_sb[:, t % G, (t // G) * QC : (t // G + 1) * QC]
                    s_ps = psA.tile([128, QC], F32, tag="s")
                    nc.tensor.matmul(
                        s_ps[:],
                        kt_sb[:, kt * 128 : (kt + 1) * 128],
                        qsl, start=True, stop=True,
                    )
                    nc.scalar.activation(
                        ep[:, kt, :], s_ps[:],
                        mybir.ActivationFunctionType.Exp, scale=SCALE,
                    )
                    if kt % 2 == 1:
                        # kt-tree level 1 on DVE
                        tr = tr_pool.tile([128, QC], BF16, tag="t1", bufs=9)
                        nc.vector.tensor_add(
                            tr[:], ep[:, kt - 1, :], ep[:, kt, :]
                        )
                        tr1.append(tr)

                ob_blk = {}

                def emit_y_group(b, no):
                    qc_o, si = b // 4, b % 4
                    if si == 0 and no == 0:
                        rs_in = dram.tile([QC, HID], BF16, tag=f"rsin{qc_o}", name="rs_in")
                        rs_out = dram.tile([RROWS, HID], BF16, tag=f"rsout{qc_o}", name="rs_out")
                        rs_tiles[qc_o] = (rs_in, rs_out)
                    if no == 0:
                        ob_blk[b] = ob_pool.tile([128, HID], BF16, tag="ob", name="ob")
                    rs_in, rs_out = rs_tiles[qc_o]
                    ob = ob_blk[b]
                    y_ps = psY.tile([128, 512], F32, tag="y")
                    for h in range(G):
                        nc.tensor.matmul(
                            y_ps[:],
                            ot_tiles[qc_o * G + h][:, si * ST : (si + 1) * ST],
                            wo_sb[:, h, no * 512 : (no + 1) * 512],
                            start=(h == 0), stop=(h == G - 1),
                        )
                    nc.vector.tensor_copy(ob[:, no * 512 : (no + 1) * 512], y_ps[:])
                    nc.sync.dma_start(
                        rs_in[si * ST : (si + 1) * ST, no * 512 : (no + 1) * 512],
                        ob[:, no * 512 : (no + 1) * 512],
                    )
                    if si == 3 and no == NO - 1:
                        orow = qc_o * RROWS
                        if single:
                            nc.sync.dma_start(
                                out_d[orow : orow + RROWS, :], rs_in[0:RROWS, :]
                            )
                        else:
                            nc.gpsimd.collective_compute(
                                "ReduceScatter",
                                mybir.AluOpType.add,
                                replica_groups=[list(range(NC))],
                                ins=[rs_in.opt()],
                                outs=[rs_out.opt()],
                            )
                            nc.sync.dma_start(
                                out_d[orow : orow + RROWS, :], rs_out[:]
                            )

                Y_KT = {2: 0, 5: 1, 8: 2, 11: 3, 14: 4}  # kt -> o_proj no group
                for t in range(NU + 2):
                    if t < NU:
                        ep = ep_pool.tile([128, N_KT, QC], BF16, tag="ep")
                        unit_state[t] = [ep, None, []]
                    if 1 <= t <= NU:
                        unit_state[t - 1][1] = psB.tile(
                            [128, QC], F32, tag="b", name="pv_ps"
                        )
                    block = t - 5 if 5 <= t < NU + 1 else None
                    # ---- per-kt interleave ----
                    if t <= NU:
                        for kt in range(N_KT):
                            if t < NU:
                                emit_scores_kt(t, kt)
                            if t >= 1:
                                epm, pvm = unit_state[t - 1][:2]
                                nc.tensor.matmul(
                                    pvm[:], v_sb[:, kt, :], epm[:, kt, :],
                                    start=(kt == 0), stop=(kt == N_KT - 1),
                                )
                            if block is not None and kt in Y_KT:
                                emit_y_group(block, Y_KT[kt])
                    # ---- finish unit u_{t-1}: ones-mm, recip, ot ----
                    if 1 <= t <= NU:
                        epm, pvm, _, colsum = unit_state[t - 1]
                        sums_ps = psY.tile([128, QC], F32, tag="y", name="sums_ps")
                        nc.tensor.matmul(
                            sums_ps[:], ones_k[:], colsum[:], start=True, stop=True
                        )
                        rb = sm_pool.tile([128, QC], F32, tag="rb", bufs=2)
                        nc.vector.reciprocal_approx_fast(rb[:], sums_ps[:])
                        ot = ot_pool.tile([128, QC], BF16, tag="ot")
                        nc.vector.tensor_tensor(
                            ot[:], pvm[:], rb[:], op=mybir.AluOpType.mult
                        )
                        ot_tiles[t - 1] = ot
                    # ---- epilogue: remaining o_proj blocks of the last chunk ----
                    if t == NU + 1:
                        for b in range(NU - 4, NU):
                            for no in range(NO):
                                emit_y_group(b, no)
                    # ---- kt-tree levels 2..4 of u_t on gpsimd ----
                    if t < NU:
                        tr1 = unit_state[t][2]
                        tr2 = []
                        for j in range(4):
                            tr = tr_pool.tile([128, QC], BF16, tag="t2", bufs=5)
                            nc.gpsimd.tensor_add(tr[:], tr1[2 * j][:], tr1[2 * j + 1][:])
                            tr2.append(tr)
                        tr3 = []
                        for j in range(2):
                            tr = tr_pool.tile([128, QC], BF16, tag="t3", bufs=4)
                            nc.gpsimd.tensor_add(tr[:], tr2[2 * j][:], tr2[2 * j + 1][:])
                            tr3.append(tr)
                        colsum = tr_pool.tile([128, QC], BF16, tag="t4", bufs=4)
                        nc.gpsimd.tensor_add(colsum[:], tr3[0][:], tr3[1][:])
                        unit_state[t].append(colsum)

    nc.compile()
    return nc


def _get_nc():
    global _NC_CACHE
    if _NC_CACHE is None:
        _NC_CACHE = _build()
    return _NC_CACHE


def make_in_maps(inputs):
    X = np.asarray(inputs["hidden_states"], dtype=np.float32).reshape(S, HID)
    freqs = np.asarray(inputs["freqs_cis"], dtype=np.float32)
    Wq = np.asarray(inputs["Wq"], dtype=np.float32)
    Wk = np.asarray(inputs["Wk"], dtype=np.float32)
    Wv = np.asarray(inputs["Wv"], dtype=np.float32)
    Wo = np.asarray(inputs["Wo"], dtype=np.float32)
    qw = np.asarray(inputs["q_norm_w"], dtype=np.float32)
    kw = np.asarray(inputs["k_norm_w"], dtype=np.float32)

    bf = ml_dtypes.bfloat16
    # X^T load tiles: (L, ch, p, s) = X[L*XL+s, ch*128+p]
    xt = np.ascontiguousarray(
        X.reshape(N_XL, XL, HC, 128).transpose(0, 2, 3, 1).astype(bf)
    )
    cos, sin = freqs[0], freqs[1]  # [S, D]
    cwq = np.ascontiguousarray((cos * qw[None, :]).reshape(N_ST, 128, D))
    swq = np.ascontiguousarray((sin * np.roll(qw, D // 2)[None, :]).reshape(N_ST, 128, D))
    cwk = np.ascontiguousarray((cos * kw[None, :]).reshape(N_ST, 128, D))
    swk = np.ascontiguousarray((sin * np.roll(kw, D // 2)[None, :]).reshape(N_ST, 128, D))

    in_maps = []
    for c in range(NC):
        wq_c = Wq[c * DQ : (c + 1) * DQ, :]  # [DQ, HID]
        wq_t = np.ascontiguousarray(wq_c.T.reshape(HC, 128, DQ).astype(bf))
        wk_c = Wk[c * D : (c + 1) * D, :]
        wv_c = Wv[c * D : (c + 1) * D, :]
        wkv_t = np.ascontiguousarray(
            np.concatenate([wk_c.T, wv_c.T], axis=1).reshape(HC, 128, 2 * D).astype(bf)
        )
        wo_c = Wo[:, c * DQ : (c + 1) * DQ]  # [HID, DQ]
        wo_t = np.ascontiguousarray(wo_c.T.reshape(G, 128, HID).astype(bf))
        in_maps.append(
            {
                "xt": xt,
                "wq": wq_t,
                "wkv": wkv_t,
                "wo": wo_t,
                "cwq": cwq,
                "swq": swq,
                "cwk": cwk,
                "swk": swk,
            }
        )
    return in_maps


def assemble(outs):
    # outs[c] is [S//NC, HID] bf16. RS chunk qc covers global rows
    # [512*qc, +512); core c receives rows [64*c, 64*c+64) of it,
    # stored at core-local rows [64*qc, +64).
    y = np.empty((S, HID), dtype=np.float32)
    rows = QC // NC  # 64
    for qc in range(N_QC):
        for c in range(NC):
            g0 = QC * qc + rows * c
            l0 = rows * qc
            y[g0 : g0 + rows, :] = outs[c][l0 : l0 + rows, :].astype(np.float32)
    return y.reshape(B, S, HID)


def kernel(**inputs) -> np.ndarray:
    nc = _get_nc()
    in_maps = make_in_maps(inputs)
    res = bass_utils.run_bass_kernel_spmd(nc, in_maps, core_ids=list(range(NC)))
    return assemble([r["out"] for r in res.results])


# revision 21
# speedup vs baseline: 1.1178x; 1.0616x over previous
# GQA attention layer (B=1, S=2048, HID=2560, H=32, HKV=8, D=128) on 8 TRN2
# NeuronCores. Tensor-parallel over kv-head groups: core c owns kv head c and
# its 4 query heads (Wq/Wk/Wv row shards, Wo column shard). The o_proj
# partials are combined with an on-device ReduceScatter over the sequence
# axis; the host reassembles the sequence-sharded outputs.
#
# Per-core dataflow (all matmuls bf16 -> fp32 PSUM):
#   1. QKV projection from X^T tiles (s-major output layout), per-head
#      RMSNorm + RoPE on DVE, PE-transpose of Q/K into [d, s] layout.
#   2. Scores are computed transposed (S^T[k, q] = K Q^T) so that the
#      P^T @ V matmul needs no transpose of the 16.8M-element prob matrix.
#      exp() on the scalar engine in 2-PSUM-bank batches (no max
#      subtraction: |scores| is bounded). Scores and PV matmuls are
#      software-pipelined so the PE never waits on the exp stream.
#      Softmax denominators: pairwise kt-tree accumulation of the exp
#      tiles on DVE, then a single all-ones [128,128] stationary matmul
#      per unit broadcasts the partition-sums to every output row; one
#      DVE reciprocal + multiply normalizes the PV output.
#   3. o_proj per 512-row chunk (DVE evictions only - the scalar engine
#      stays exp-only in phase 2, avoiding ACT table reloads). The first
#      3 chunks ReduceScatter at 512 rows; the last chunk scatters per
#      128-row subtile so the final RS tail is ~4x shorter.
import sys

if "/opt/trn_rl_repo" not in sys.path:
    sys.path.insert(0, "/opt/trn_rl_repo")

import numpy as np
import ml_dtypes

import concourse.bacc as bacc
import concourse.mybir as mybir
import concourse.tile as tile
from concourse import bass_utils, masks

BF16 = mybir.dt.bfloat16
F32 = mybir.dt.float32

B, S, HID = 1, 2048, 2560
H, HKV, D = 32, 8, 128
G = H // HKV  # q heads per kv head (= per core)
NC = 8  # cores
DQ = G * D  # per-core q width (512)
EPS = 1e-6
SCALE = 1.0 / float(np.sqrt(D))

ST = 128          # s positions per compute tile
N_ST = S // ST    # 16
HC = HID // 128   # 20 contraction chunks
XL = 256          # s positions per X^T DMA load tile
N_XL = S // XL    # 8
QC = 512          # q positions per attention unit
N_QC = S // QC    # 4
N_KT = S // 128   # 16 k tiles per attention unit
NP = N_KT // 2    # 8 score/exp pairs per unit
NO = HID // 512   # 5 o_proj free-dim chunks
MR = 128          # rows per mini-ReduceScatter (last chunk)

_NC_CACHE = None


def _build(reps: int = 1, single: bool = False):
    nc = bacc.Bacc(
        "TRN2", target_bir_lowering=False, debug=False,
        num_devices=(1 if single else NC),
    )

    xt_d = nc.dram_tensor("xt", [N_XL, HC, 128, XL], BF16, kind="ExternalInput").ap()
    wq_d = nc.dram_tensor("wq", [HC, 128, DQ], BF16, kind="ExternalInput").ap()
    wkv_d = nc.dram_tensor("wkv", [HC, 128, 2 * D], BF16, kind="ExternalInput").ap()
    wo_d = nc.dram_tensor("wo", [G, 128, HID], BF16, kind="ExternalInput").ap()
    cwq_d = nc.dram_tensor("cwq", [N_ST, 128, D], F32, kind="ExternalInput").ap()
    swq_d = nc.dram_tensor("swq", [N_ST, 128, D], F32, kind="ExternalInput").ap()
    cwk_d = nc.dram_tensor("cwk", [N_ST, 128, D], F32, kind="ExternalInput").ap()
    swk_d = nc.dram_tensor("swk", [N_ST, 128, D], F32, kind="ExternalInput").ap()
    out_d = nc.dram_tensor("out", [S // NC, HID], BF16, kind="ExternalOutput").ap()

    with tile.TileContext(nc) as tc:
        with (
            tc.tile_pool(name="const", bufs=1) as cpool,
            tc.tile_pool(name="xt", bufs=2) as xt_pool,
            tc.tile_pool(name="cs", bufs=8) as cs_pool,
            tc.tile_pool(name="qw", bufs=5) as qw_pool,
            tc.tile_pool(name="kw", bufs=6) as kw_pool,
            tc.tile_pool(name="ro", bufs=2) as ro_pool,
            tc.tile_pool(name="sm", bufs=4) as sm_pool,
            tc.tile_pool(name="ep", bufs=2) as ep_pool,
            tc.tile_pool(name="tr", bufs=2) as tr_pool,
            tc.tile_pool(name="ot", bufs=8) as ot_pool,
            tc.tile_pool(name="ob", bufs=3) as ob_pool,
            tc.tile_pool(name="psA", bufs=4, space="PSUM") as psA,
            tc.tile_pool(name="psB", bufs=2, space="PSUM") as psB,
            tc.tile_pool(name="psY", bufs=2, space="PSUM") as psY,
            tc.tile_pool(name="dram", bufs=1, space="DRAM") as dram,
        ):
            for _rep in range(reps):
                # ---- resident constants / weights ----
                ident = cpool.tile([128, 128], BF16, tag="ident")
                masks.make_identity(nc, ident[:])
                # all-ones stationary: one sums matmul per unit yields the
                # softmax denominator replicated across all 128 partitions
                ones_k = cpool.tile([128, 128], BF16, tag="ones_k")
                nc.vector.memset(ones_k[:], 1.0)

                # first two st-tiles' cos/sin tables ahead of the 5 MB
                # weight block so the st=0 norm/rope never waits on them
                cs_pre = []
                for st in range(2):
                    row = []
                    for src in (cwq_d, swq_d, cwk_d, swk_d):
                        ct = cs_pool.tile([128, D], F32, tag="cs", name="ct")
                        nc.sync.dma_start(ct[:], src[st])
                        row.append(ct)
                    cs_pre.append(row)

                # interleave per-chunk weight + xt[0] chunk loads so the
                # first st=0 matmuls start as soon as chunk 0 lands
                xt_t = xt_pool.tile([128, HC, XL], BF16, tag="xt")
                wq_t = []
                wkv_t = []
                for ch in range(HC):
                    w1 = cpool.tile([128, DQ], BF16, tag=f"wq{ch}")
                    nc.sync.dma_start(w1[:], wq_d[ch])
                    wq_t.append(w1)
                    w2 = cpool.tile([128, 2 * D], BF16, tag=f"wkv{ch}")
                    nc.sync.dma_start(w2[:], wkv_d[ch])
                    wkv_t.append(w2)
                    nc.sync.dma_start(xt_t[:, ch, :], xt_d[0, ch])
                xt_next = xt_pool.tile([128, HC, XL], BF16, tag="xt")
                nc.sync.dma_start(xt_next[:], xt_d[1].rearrange("c p s -> p c s"))

                qt_sb = cpool.tile([128, G, S], BF16, tag="qt")   # Q^T  [d, h, s]
                kt_sb = cpool.tile([128, S], BF16, tag="kt")      # K^T  [d, s]
                v_sb = cpool.tile([128, N_KT, D], BF16, tag="v")  # V    [s%128, kt, d]

                # ================= phase 1: QKV + norm + rope + transpose ======
                for st in range(N_ST):
                    if st % (XL // ST) == 0 and st > 0:
                        if st // (XL // ST) == 1:
                            xt_t = xt_next
                        else:
                            xt_t = xt_pool.tile([128, HC, XL], BF16, tag="xt")
                            nc.sync.dma_start(
                                xt_t[:],
                                xt_d[st // (XL // ST)].rearrange("c p s -> p c s"),
                            )
                    soff = (st % (XL // ST)) * ST

                    if st < 2:
                        cwq_t, swq_t, cwk_t, swk_t = cs_pre[st]
                    else:
                        cwq_t = cs_pool.tile([128, D], F32, tag="cs")
                        nc.sync.dma_start(cwq_t[:], cwq_d[st])
                        swq_t = cs_pool.tile([128, D], F32, tag="cs")
                        nc.sync.dma_start(swq_t[:], swq_d[st])
                        cwk_t = cs_pool.tile([128, D], F32, tag="cs")
                        nc.sync.dma_start(cwk_t[:], cwk_d[st])
                        swk_t = cs_pool.tile([128, D], F32, tag="cs")
                        nc.sync.dma_start(swk_t[:], swk_d[st])

                    q_ps = psA.tile([128, DQ], F32, tag="s")
                    kv_ps = psA.tile([128, DQ], F32, tag="s")
                    for ch in range(HC):
                        lhs = xt_t[:, ch, soff : soff + ST]
                        nc.tensor.matmul(
                            q_ps[:], lhs, wq_t[ch][:],
                            start=(ch == 0), stop=(ch == HC - 1),
                        )
                        nc.tensor.matmul(
                            kv_ps[:, 0 : 2 * D], lhs, wkv_t[ch][:],
                            start=(ch == 0), stop=(ch == HC - 1),
                        )

                    # evictions (scalar engine; phase 1 is Copy-table only)
                    q_sb = qw_pool.tile([128, DQ], F32, tag="qw")
                    nc.scalar.copy(q_sb[:], q_ps[:])
                    k_sb = kw_pool.tile([128, D], F32, tag="kw")
                    nc.scalar.copy(k_sb[:], kv_ps[:, 0:D])
                    nc.scalar.copy(v_sb[:, st, :], kv_ps[:, D : 2 * D])

                    # ---- RMSNorm (per head) ----
                    sq = qw_pool.tile([128, DQ], F32, tag="qw")
                    nc.vector.tensor_mul(sq[:], q_sb[:], q_sb[:])
                    ssq = sm_pool.tile([128, G + 1], F32, tag="sm")
                    nc.vector.tensor_reduce(
                        ssq[:, 0:G], sq[:].rearrange("p (h d) -> p h d", d=D),
                        axis=mybir.AxisListType.X, op=mybir.AluOpType.add,
                    )
                    ksq = kw_pool.tile([128, D], F32, tag="kw")
                    nc.vector.tensor_mul(ksq[:], k_sb[:], k_sb[:])
                    nc.vector.tensor_reduce(
                        ssq[:, G : G + 1], ksq[:].unsqueeze(1),
                        axis=mybir.AxisListType.X, op=mybir.AluOpType.add,
                    )
                    var = sm_pool.tile([128, G + 1], F32, tag="sm")
                    nc.vector.tensor_scalar(
                        var[:], ssq[:], 1.0 / D, EPS,
                        op0=mybir.AluOpType.mult, op1=mybir.AluOpType.add,
                    )
                    rt = sm_pool.tile([128, G + 1], F32, tag="sm")
                    nc.scalar.sqrt(rt[:], var[:])
                    rq = sm_pool.tile([128, G + 1], F32, tag="sm")
                    nc.vector.reciprocal(rq[:], rt[:])
                    rk = rq

                    # ---- normalize + rope (DVE) ----
                    qn = qw_pool.tile([128, DQ], F32, tag="qw")
                    qn3 = qn[:].rearrange("p (h d) -> p h d", d=D)
                    nc.vector.tensor_tensor(
                        qn3, q_sb[:].rearrange("p (h d) -> p h d", d=D),
                        rq[:, 0:G].unsqueeze(2).to_broadcast([128, G, D]),
                        op=mybir.AluOpType.mult,
                    )
                    t1 = qw_pool.tile([128, DQ], F32, tag="qw")
                    t13 = t1[:].rearrange("p (h d) -> p h d", d=D)
                    cwq3 = cwq_t[:].unsqueeze(1).to_broadcast([128, G, D])
                    swq3 = swq_t[:].unsqueeze(1).to_broadcast([128, G, D])
                    nc.vector.tensor_tensor(t13, qn3, cwq3, op=mybir.AluOpType.mult)
                    u = qw_pool.tile([128, DQ], F32, tag="qw")
                    u3 = u[:].rearrange("p (h d) -> p h d", d=D)
                    hd = D // 2
                    nc.vector.tensor_tensor(
                        u3[:, :, 0:hd], qn3[:, :, hd:D], swq3[:, :, 0:hd],
                        op=mybir.AluOpType.mult,
                    )
                    nc.vector.tensor_tensor(
                        u3[:, :, hd:D], qn3[:, :, 0:hd], swq3[:, :, hd:D],
                        op=mybir.AluOpType.mult,
                    )
                    qro = ro_pool.tile([128, DQ], BF16, tag="qro")
                    qro3 = qro[:].rearrange("p (h d) -> p h d", d=D)
                    nc.vector.tensor_sub(qro3[:, :, 0:hd], t13[:, :, 0:hd], u3[:, :, 0:hd])
                    nc.vector.tensor_add(qro3[:, :, hd:D], t13[:, :, hd:D], u3[:, :, hd:D])

                    kn = kw_pool.tile([128, D], F32, tag="kw")
                    nc.vector.tensor_tensor(
                        kn[:], k_sb[:],
                        rk[:, G : G + 1].to_broadcast([128, D]),
                        op=mybir.AluOpType.mult,
                    )
                    kt1 = kw_pool.tile([128, D], F32, tag="kw")
                    nc.vector.tensor_tensor(kt1[:], kn[:], cwk_t[:], op=mybir.AluOpType.mult)
                    ku = kw_pool.tile([128, D], F32, tag="kw")
                    nc.vector.tensor_tensor(
                        ku[:, 0:hd], kn[:, hd:D], swk_t[:, 0:hd], op=mybir.AluOpType.mult
                    )
                    nc.vector.tensor_tensor(
                        ku[:, hd:D], kn[:, 0:hd], swk_t[:, hd:D], op=mybir.AluOpType.mult
                    )
                    kro = ro_pool.tile([128, D], BF16, tag="kro")
                    nc.vector.tensor_sub(kro[:, 0:hd], kt1[:, 0:hd], ku[:, 0:hd])
                    nc.vector.tensor_add(kro[:, hd:D], kt1[:, hd:D], ku[:, hd:D])

                    # ---- transpose Q heads + K into [d, s] ----
                    for h in range(G):
                        tp = psY.tile([128, 128], BF16, tag="y", name="tp")
                        nc.tensor.transpose(tp[:], qro[:, h * D : (h + 1) * D], ident[:])
                        nc.scalar.copy(qt_sb[:, h, st * ST : (st + 1) * ST], tp[:])
                    tp = psY.tile([128, 128], BF16, tag="y")
                    nc.tensor.transpose(tp[:], kro[:], ident[:])
                    nc.scalar.copy(kt_sb[:, st * ST : (st + 1) * ST], tp[:])

                # ================= phase 2: attention + o_proj + RS ============
                # wo is first needed ~20us into phase 2; load it behind the
                # phase-1 traffic instead of ahead of it
                wo_sb = cpool.tile([128, G, HID], BF16, tag="wo")
                nc.sync.dma_start(wo_sb[:], wo_d.rearrange("c p n -> p c n"))

                # Slot pipeline over 16 attention units u = (qc, h):
                #   slot t: per-kt interleave of scores+exp+L1(u_t), pv(u_{t-1}),
                #   and the y-groups of one o_proj 128-row si-block (chunk-lagged
                #   by 1 slot past the chunk boundary so the last ot is ready).
                #   ones/recip/ot(u_{t-1}) at slot end; tree L2-4 of u_t on
                #   gpsimd (latency-tolerant: result needed a full slot later).
                #   The exp stream (scalar) and the sums tree never gate the PE;
                #   ob bufs=3 + psY bufs=2 ride out DMA outages while RS wire
                #   traffic hogs the queues.
                NU = N_QC * G  # 16 units
                RROWS = QC // NC  # 64 output rows per core per RS chunk
                unit_state = [None] * NU  # [ep, pv_ps, tr1, colsum]
                ot_tiles = {}
                rs_tiles = {}

                def emit_scores_kt(t, kt):
                    ep, _, tr1 = unit_state[t][:3]
                    qsl = qt_sb[:, t % G, (t // G) * QC : (t // G + 1) * QC]
                    s_ps = psA.tile([128, QC], F32, tag="s")
                    nc.tensor.matmul(
                        s_ps[:],
                        kt_sb[:, kt * 128 : (kt + 1) * 128],
                        qsl, start=True, stop=True,
                    )
                    nc.scalar.activation(
                        ep[:, kt, :], s_ps[:],
                        mybir.ActivationFunctionType.Exp, scale=SCALE,
                    )
                    if kt % 2 == 1:
                        # kt-tree level 1 on DVE
                        tr = tr_pool.tile([128, QC], BF16, tag="t1", bufs=9)
                        nc.vector.tensor_add(
                            tr[:], ep[:, kt - 1, :], ep[:, kt, :]
                        )
                        tr1.append(tr)

                ob_blk = {}

                def emit_y_group(b, no):
                    qc_o, si = b // 4, b % 4
                    if si == 0 and no == 0:
                        rs_in = dram.tile([QC, HID], BF16, tag=f"rsin{qc_o}", name="rs_in")
                        rs_out = dram.tile([RROWS, HID], BF16, tag=f"rsout{qc_o}", name="rs_out")
                        rs_tiles[qc_o] = (rs_in, rs_out)
                    if no == 0:
                        ob_blk[b] = ob_pool.tile([128, HID], BF16, tag="ob", name="ob")
                    rs_in, rs_out = rs_tiles[qc_o]
                    ob = ob_blk[b]
                    y_ps = psY.tile([128, 512], F32, tag="y")
                    for h in range(G):
                        nc.tensor.matmul(
                            y_ps[:],
                            ot_tiles[qc_o * G + h][:, si * ST : (si + 1) * ST],
                            wo_sb[:, h, no * 512 : (no + 1) * 512],
                            start=(h == 0), stop=(h == G - 1),
                        )
                    nc.vector.tensor_copy(ob[:, no * 512 : (no + 1) * 512], y_ps[:])
                    nc.sync.dma_start(
                        rs_in[si * ST : (si + 1) * ST, no * 512 : (no + 1) * 512],
                        ob[:, no * 512 : (no + 1) * 512],
                    )
                    if si == 3 and no == NO - 1:
                        orow = qc_o * RROWS
                        if single:
                            nc.sync.dma_start(
                                out_d[orow : orow + RROWS, :], rs_in[0:RROWS, :]
                            )
                        else:
                            nc.gpsimd.collective_compute(
                                "ReduceScatter",
                                mybir.AluOpType.add,
                                replica_groups=[list(range(NC))],
                                ins=[rs_in.opt()],
                                outs=[rs_out.opt()],
                            )
                            nc.sync.dma_start(
                                out_d[orow : orow + RROWS, :], rs_out[:]
                            )

                Y_KT = {2: 0, 5: 1, 8: 2, 11: 3, 14: 4}  # kt -> o_proj no group
                for t in range(NU + 2):
                    if t < NU:
                        ep = ep_pool.tile([128, N_KT, QC], BF16, tag="ep")
                        unit_state[t] = [ep, None, []]
                    if 1 <= t <= NU:
                        unit_state[t - 1][1] = psB.tile(
                            [128, QC], F32, tag="b", name="pv_ps"
                        )
                    # si=1..3 blocks interleave into the kt loop; si=0 blocks
                    # append at the end of slot 4qc+4 (their last ot input is
                    # computed mid-slot), so RS(qc) fires at slot 4qc+7
                    block = None
                    if 5 <= t <= 15 and (t - 4) % 4 != 0:
                        block = t - 4
                    # ---- per-kt interleave ----
                    if t <= NU:
                        for kt in range(N_KT):
                            if t < NU:
                                emit_scores_kt(t, kt)
                            if t >= 1:
                                epm, pvm = unit_state[t - 1][:2]
                                nc.tensor.matmul(
                                    pvm[:], v_sb[:, kt, :], epm[:, kt, :],
                                    start=(kt == 0), stop=(kt == N_KT - 1),
                                )
                            if block is not None and kt in Y_KT:
                                emit_y_group(block, Y_KT[kt])
                    # ---- finish unit u_{t-1}: ones-mm, recip, ot ----
                    if 1 <= t <= NU:
                        epm, pvm, _, colsum = unit_state[t - 1]
                        sums_ps = psY.tile([128, QC], F32, tag="y", name="sums_ps")
                        nc.tensor.matmul(
                            sums_ps[:], ones_k[:], colsum[:], start=True, stop=True
                        )
                        rb = sm_pool.tile([128, QC], F32, tag="rb", bufs=2)
                        nc.vector.reciprocal_approx_fast(rb[:], sums_ps[:])
                        ot = ot_pool.tile([128, QC], BF16, tag="ot")
                        nc.vector.tensor_tensor(
                            ot[:], pvm[:], rb[:], op=mybir.AluOpType.mult
                        )
                        ot_tiles[t - 1] = ot
                    # ---- si=0 block of chunk (t-4)//4 appended at slot end ----
                    if t in (4, 8, 12, 16):
                        for no in range(NO):
                            emit_y_group(t - 4, no)
                    # ---- epilogue: last chunk's si=1..3 blocks ----
                    if t == NU + 1:
                        for b in range(NU - 3, NU):
                            for no in range(NO):
                                emit_y_group(b, no)
                    # ---- kt-tree levels 2..4 of u_t on gpsimd ----
                    if t < NU:
                        tr1 = unit_state[t][2]
                        tr2 = []
                        for j in range(4):
                            tr = tr_pool.tile([128, QC], BF16, tag="t2", bufs=5)
                            nc.gpsimd.tensor_add(tr[:], tr1[2 * j][:], tr1[2 * j + 1][:])
                            tr2.append(tr)
                        tr3 = []
                        for j in range(2):
                            tr = tr_pool.tile([128, QC], BF16, tag="t3", bufs=4)
                            nc.gpsimd.tensor_add(tr[:], tr2[2 * j][:], tr2[2 * j + 1][:])
                            tr3.append(tr)
                        colsum = tr_pool.tile([128, QC], BF16, tag="t4", bufs=4)
                        nc.gpsimd.tensor_add(colsum[:], tr3[0][:], tr3[1][:])
                        unit_state[t].append(colsum)

    nc.compile()
    return nc


def _get_nc():
    global _NC_CACHE
    if _NC_CACHE is None:
        _NC_CACHE = _build()
    return _NC_CACHE


def make_in_maps(inputs):
    X = np.asarray(inputs["hidden_states"], dtype=np.float32).reshape(S, HID)
    freqs = np.asarray(inputs["freqs_cis"], dtype=np.float32)
    Wq = np.asarray(inputs["Wq"], dtype=np.float32)
    Wk = np.asarray(inputs["Wk"], dtype=np.float32)
    Wv = np.asarray(inputs["Wv"], dtype=np.float32)
    Wo = np.asarray(inputs["Wo"], dtype=np.float32)
    qw = np.asarray(inputs["q_norm_w"], dtype=np.float32)
    kw = np.asarray(inputs["k_norm_w"], dtype=np.float32)

    bf = ml_dtypes.bfloat16
    # X^T load tiles: (L, ch, p, s) = X[L*XL+s, ch*128+p]
    xt = np.ascontiguousarray(
        X.reshape(N_XL, XL, HC, 128).transpose(0, 2, 3, 1).astype(bf)
    )
    cos, sin = freqs[0], freqs[1]  # [S, D]
    cwq = np.ascontiguousarray((cos * qw[None, :]).reshape(N_ST, 128, D))
    swq = np.ascontiguousarray((sin * np.roll(qw, D // 2)[None, :]).reshape(N_ST, 128, D))
    cwk = np.ascontiguousarray((cos * kw[None, :]).reshape(N_ST, 128, D))
    swk = np.ascontiguousarray((sin * np.roll(kw, D // 2)[None, :]).reshape(N_ST, 128, D))

    in_maps = []
    for c in range(NC):
        wq_c = Wq[c * DQ : (c + 1) * DQ, :]  # [DQ, HID]
        wq_t = np.ascontiguousarray(wq_c.T.reshape(HC, 128, DQ).astype(bf))
        wk_c = Wk[c * D : (c + 1) * D, :]
        wv_c = Wv[c * D : (c + 1) * D, :]
        wkv_t = np.ascontiguousarray(
            np.concatenate([wk_c.T, wv_c.T], axis=1).reshape(HC, 128, 2 * D).astype(bf)
        )
        wo_c = Wo[:, c * DQ : (c + 1) * DQ]  # [HID, DQ]
        wo_t = np.ascontiguousarray(wo_c.T.reshape(G, 128, HID).astype(bf))
        in_maps.append(
            {
                "xt": xt,
                "wq": wq_t,
                "wkv": wkv_t,
                "wo": wo_t,
                "cwq": cwq,
                "swq": swq,
                "cwk": cwk,
                "swk": swk,
            }
        )
    return in_maps


def assemble(outs):
    # outs[c] is [S//NC, HID] bf16. RS chunk qc covers global rows
    # [512*qc, +512); core c receives rows [64*c, 64*c+64) of it,
    # stored at core-local rows [64*qc, +64).
    y = np.empty((S, HID), dtype=np.float32)
    rows = QC // NC  # 64
    for qc in range(N_QC):
        for c in range(NC):
            g0 = QC * qc + rows * c
            l0 = rows * qc
            y[g0 : g0 + rows, :] = outs[c][l0 : l0 + rows, :].astype(np.float32)
    return y.reshape(B, S, HID)


def kernel(**inputs) -> np.ndarray:
    nc = _get_nc()
    in_maps = make_in_maps(inputs)
    res = bass_utils.run_bass_kernel_spmd(nc, in_maps, core_ids=list(range(NC)))
    return assemble([r["out"] for r in res.results])
